# revision 11
# baseline (speedup 1.0000x reference)
"""BatchConv2D (per-sample-weight conv) Trainium2 Bass kernel.

Problem: x [16,4,64,64,64], weight [16,128,64,3,3], bias [16,128] (all f32)
out[bi,bj] = conv2d(x[bi,bj], weight[bi], pad=1) + bias[bi]  -> [16,4,128,64,64]

Sharding: b_i axis split across 8 cores (2 per core); no communication.

Per-core kernel strategy ("parity-split" conv-as-matmul):
  - Each image is stored in SBUF as [128 partitions, 33, 66]:
      partitions   0..63  = channel c, EVEN padded input rows (0,2,..,64)
      partitions  64..127 = channel c, ODD  padded input rows (-1,1,..,63)
    with the 1-pixel zero border baked in host-side so the load DMA is one
    fully-contiguous 1.1MB transfer.
  - A 3x3 conv tap (ky,kx) restricted to output rows of a fixed parity q
    reads input rows of a single parity -> a K=64 matmul sourced from one
    partition half. Interleaving matmuls whose sources alternate partition
    halves lets the PE run them concurrently in different row-groups: full
    128-row array utilization with zero data duplication.
  - A PSUM accumulation group must keep a single base partition (mixing
    halves in one chain faults the device), so each 16-row output group
    uses 4 PSUM banks: (q=0, ky in {0,2}):half1, (q=0, ky=1):half0+bias,
    (q=1, ky in {0,2}):half0, (q=1, ky=1):half1+bias. The bias rides in as
    a K=1 matmul against a ones vector; DVE merges each bank pair.
  - float32r dtype: fp32 with 11 stored mantissa bits at 1 cycle/row (4x
    faster than the plain fp32 matmul path). Inputs are pre-rounded
    (round-to-nearest-even) host-side so the PE's truncation is exact.
"""

import numpy as np

B_I, B_J, C, H, W = 16, 4, 64, 64, 64
OC, KH, KW = 128, 3, 3
N_CORES = 8
BPC = B_I // N_CORES          # b_i per core
NIMG = BPC * B_J              # images per core
RH = H // 2 + 1               # 33 rows per parity half (padded)
WP = W + 2                    # 66 padded width
GROUPS = 4                    # output row-groups of 16 rows per image
RG = H // GROUPS // 2         # 8 r'' rows per parity per group

_CACHE = {}


def _build_nc(repeat=1):
    import concourse.mybir as mybir
    from concourse import bacc, tile

    F32 = mybir.dt.float32
    F32R = mybir.dt.float32r

    nc = bacc.Bacc("TRN2", target_bir_lowering=False, debug=False)
    x_d = nc.dram_tensor("x", [NIMG, 2 * C, RH, WP], F32R, kind="ExternalInput")
    wt_d = nc.dram_tensor("wt", [BPC, 2 * C, KH * KW, OC], F32R, kind="ExternalInput")
    b_d = nc.dram_tensor("bias", [2, BPC * OC], F32R, kind="ExternalInput")
    ones_d = nc.dram_tensor("ones", [2, RG * W], F32R, kind="ExternalInput")
    o_d = nc.dram_tensor("out", [BPC, B_J, OC, H, W], F32, kind="ExternalOutput")

    with tile.TileContext(nc) as tc:
        with (
            tc.tile_pool(name="const", bufs=1) as cpool,
            tc.tile_pool(name="img", bufs=1) as ipool,
            tc.tile_pool(name="osb", bufs=1) as opool,
            tc.tile_pool(name="ps", bufs=1, space="PSUM") as pspool,
        ):
            wt_t = []
            for bi in range(BPC):
                w = cpool.tile([2 * C, KH * KW, OC], F32R, name=f"wt{bi}", tag=f"wt{bi}")
                nc.sync.dma_start(w[:, :, :], wt_d[bi])
                wt_t.append(w)
            # bias rows live on partitions 0 and 64 so the K=1 bias matmul
            # can source from either PE row-group half
            biasrow = cpool.tile([2 * C, BPC * OC], F32R, name="biasrow", tag="biasrow")
            nc.sync.dma_start(biasrow[0:1, :], b_d[0:1, :])
            nc.sync.dma_start(biasrow[64:65, :], b_d[1:2, :])
            ones = cpool.tile([2 * C, RG * W], F32R, name="ones", tag="ones")
            nc.sync.dma_start(ones[0:1, :], ones_d[0:1, :])
            nc.sync.dma_start(ones[64:65, :], ones_d[1:2, :])

            for rep in range(repeat):
              for bi in range(BPC):
                for bj in range(B_J):
                    idx = bi * B_J + bj
                    img = ipool.tile(
                        [2 * C, RH, WP], F32R, name="img", tag="img", bufs=3
                    )
                    nc.sync.dma_start(img[:, :, :], x_d[idx])

                    for g in range(GROUPS):
                        y0 = g * (H // GROUPS)
                        # bank -> (q, taps, half): A: q0 ky0/ky2 half1;
                        # B: q0 ky1 half0 (+bias); C: q1 ky0/ky2 half0;
                        # D: q1 ky1 half1 (+bias)
                        pst = {
                            k: pspool.tile(
                                [OC, RG, W], F32, name=f"ps{k}", tag=f"ps{k}", bufs=2
                            )
                            for k in "ABCD"
                        }

                        def mm(bank, q, ky, kx, start, stop):
                            s = q + ky - 1
                            half = s & 1
                            rh0 = y0 // 2 + (1 if s >= 1 else 0)
                            nc.tensor.matmul(
                                pst[bank][:, :, :],
                                wt_t[bi][64 * half : 64 * half + 64, KW * ky + kx, :],
                                img[
                                    64 * half : 64 * half + 64,
                                    rh0 : rh0 + RG,
                                    kx : kx + W,
                                ],
                                start=start,
                                stop=stop,
                            )

                        def bias_mm(bank, half, stop):
                            nc.tensor.matmul(
                                pst[bank][:, :, :],
                                biasrow[
                                    64 * half : 64 * half + 1, bi * OC : (bi + 1) * OC
                                ],
                                ones[64 * half : 64 * half + 1, :],
                                start=False,
                                stop=stop,
                            )

                        # A/C chains: 6 taps each, halves (1,0) alternating
                        ac_taps = [(0, 0), (0, 1), (0, 2), (2, 0), (2, 1), (2, 2)]
                        for t, (ky, kx) in enumerate(ac_taps):
                            mm("A", 0, ky, kx, start=(t == 0), stop=(t == 5))
                            mm("C", 1, ky, kx, start=(t == 0), stop=(t == 5))
                        # B/D chains: 3 taps + bias, halves (0,1) alternating
                        for t, kx in enumerate((0, 1, 2)):
                            mm("B", 0, 1, kx, start=(t == 0), stop=False)
                            mm("D", 1, 1, kx, start=(t == 0), stop=False)
                        bias_mm("B", 0, stop=True)
                        bias_mm("D", 1, stop=True)

                        osb = opool.tile(
                            [OC, RG, 2, W], F32, name="osb", tag="osb", bufs=3
                        )
                        nc.vector.tensor_copy(osb[:, :, 0, :], pst["A"][:, :, :])
                        nc.vector.tensor_add(
                            osb[:, :, 0, :], osb[:, :, 0, :], pst["B"][:, :, :]
                        )
                        nc.vector.tensor_copy(osb[:, :, 1, :], pst["C"][:, :, :])
                        nc.vector.tensor_add(
                            osb[:, :, 1, :], osb[:, :, 1, :], pst["D"][:, :, :]
                        )
                        nc.scalar.dma_start(
                            o_d[bi, bj, :, y0 : y0 + H // GROUPS, :], osb[:, :, :, :]
                        )
    nc.compile()
    return nc


def _round_fp32r(a):
    """RNE-round fp32 to fp32r (11 stored mantissa bits; low 12 bits zero)."""
    u = a.view(np.uint32)
    r = (u + np.uint32(0x7FF) + ((u >> np.uint32(12)) & np.uint32(1))) & np.uint32(
        0xFFFFF000
    )
    return r.view(np.float32)


def _pack(x, weight, bias):
    """Host-side repack into the kernel's DMA-friendly layouts."""
    x = _round_fp32r(np.ascontiguousarray(x, dtype=np.float32))
    weight = _round_fp32r(np.ascontiguousarray(weight, dtype=np.float32))
    bias = _round_fp32r(np.ascontiguousarray(bias, dtype=np.float32))

    xp = np.zeros((B_I, B_J, 2, C, RH, WP), np.float32)
    xp[:, :, 0, :, 0:32, 1 : W + 1] = x[:, :, :, 0::2, :]   # even rows 0..62
    xp[:, :, 1, :, 1:33, 1 : W + 1] = x[:, :, :, 1::2, :]   # odd rows 1..63
    xp = xp.reshape(B_I, B_J, 2 * C, RH, WP)

    wt0 = np.ascontiguousarray(np.transpose(weight, (0, 2, 3, 4, 1))).reshape(
        B_I, C, KH * KW, OC
    )
    wt = np.concatenate([wt0, wt0], axis=1)  # duplicate across partition halves

    return xp, wt, bias


def kernel(x, weight, bias):
    from concourse.bass_utils import run_bass_kernel_spmd

    xp, wt, brnd = _pack(x, weight, bias)

    if "nc" not in _CACHE:
        _CACHE["nc"] = _build_nc()
    nc = _CACHE["nc"]

    in_maps = []
    for i in range(N_CORES):
        sl = slice(i * BPC, (i + 1) * BPC)
        brow = np.ascontiguousarray(
            np.broadcast_to(brnd[sl].reshape(1, BPC * OC), (2, BPC * OC))
        )
        in_maps.append(
            {
                "x": np.ascontiguousarray(xp[sl].reshape(NIMG, 2 * C, RH, WP)),
                "wt": np.ascontiguousarray(wt[sl]),
                "bias": brow,
                "ones": np.ones((2, RG * W), np.float32),
            }
        )

    res = run_bass_kernel_spmd(nc, in_maps, list(range(N_CORES)))
    out = np.concatenate([res.results[i]["out"] for i in range(N_CORES)], axis=0)
    return out


# revision 13
# speedup vs baseline: 1.1313x; 1.1313x over previous
"""BatchConv2D (per-sample-weight conv) Trainium2 Bass kernel.

Problem: x [16,4,64,64,64], weight [16,128,64,3,3], bias [16,128] (all f32)
out[bi,bj] = conv2d(x[bi,bj], weight[bi], pad=1) + bias[bi]  -> [16,4,128,64,64]

Sharding: b_i axis split across 8 cores (2 per core); no communication.

Per-core kernel strategy ("parity-split" conv-as-matmul):
  - Each image is stored in SBUF as [128 partitions, 33, 66]:
      partitions   0..63  = channel c, EVEN padded input rows (0,2,..,64)
      partitions  64..127 = channel c, ODD  padded input rows (-1,1,..,63)
    with the 1-pixel zero border baked in host-side so the load DMA is one
    fully-contiguous 1.1MB transfer.
  - A 3x3 conv tap (ky,kx) restricted to output rows of a fixed parity q
    reads input rows of a single parity -> a K=64 matmul sourced from one
    partition half. Interleaving matmuls whose sources alternate partition
    halves lets the PE run them concurrently in different row-groups: full
    128-row array utilization with zero data duplication.
  - A PSUM accumulation group must keep a single base partition (mixing
    halves in one chain faults the device), so each 16-row output group
    uses 4 PSUM banks: (q=0, ky in {0,2}):half1, (q=0, ky=1):half0+bias,
    (q=1, ky in {0,2}):half0, (q=1, ky=1):half1+bias. The bias rides in as
    a K=1 matmul against a ones vector; DVE merges each bank pair.
  - float32r dtype: fp32 with 11 stored mantissa bits at 1 cycle/row (4x
    faster than the plain fp32 matmul path). Inputs are pre-rounded
    (round-to-nearest-even) host-side so the PE's truncation is exact.
"""

import numpy as np

B_I, B_J, C, H, W = 16, 4, 64, 64, 64
OC, KH, KW = 128, 3, 3
N_CORES = 8
BPC = B_I // N_CORES          # b_i per core
NIMG = BPC * B_J              # images per core
RH = H // 2 + 1               # 33 rows per parity half (padded)
WP = W + 2                    # 66 padded width
GROUPS = 4                    # output row-groups of 16 rows per image
RG = H // GROUPS // 2         # 8 r'' rows per parity per group

_CACHE = {}


def _build_nc(repeat=1):
    import concourse.mybir as mybir
    from concourse import bacc, tile

    F32 = mybir.dt.float32
    F32R = mybir.dt.float32r

    nc = bacc.Bacc("TRN2", target_bir_lowering=False, debug=False)
    x_d = nc.dram_tensor("x", [NIMG, 2 * C, RH, WP], F32R, kind="ExternalInput")
    wt_d = nc.dram_tensor("wt", [BPC, 2 * C, KH * KW, OC], F32R, kind="ExternalInput")
    b_d = nc.dram_tensor("bias", [OC, BPC], F32, kind="ExternalInput")
    o_d = nc.dram_tensor("out", [BPC, B_J, OC, H, W], F32, kind="ExternalOutput")

    with tile.TileContext(nc) as tc:
        with (
            tc.tile_pool(name="const", bufs=1) as cpool,
            tc.tile_pool(name="img", bufs=1) as ipool,
            tc.tile_pool(name="osb", bufs=1) as opool,
            tc.tile_pool(name="ps", bufs=1, space="PSUM") as pspool,
        ):
            wt_t = []
            for bi in range(BPC):
                w = cpool.tile([2 * C, KH * KW, OC], F32R, name=f"wt{bi}", tag=f"wt{bi}")
                nc.sync.dma_start(w[:, :, :], wt_d[bi])
                wt_t.append(w)
            bias_t = cpool.tile([OC, BPC], F32, name="bias_t", tag="bias")
            nc.sync.dma_start(bias_t[:, :], b_d[:, :])

            for rep in range(repeat):
              for bi in range(BPC):
                for bj in range(B_J):
                    idx = bi * B_J + bj
                    img = ipool.tile(
                        [2 * C, RH, WP], F32R, name="img", tag="img", bufs=3
                    )
                    nc.sync.dma_start(img[:, :, :], x_d[idx])

                    for g in range(GROUPS):
                        y0 = g * (H // GROUPS)
                        # Three constant-base accumulation chains per group:
                        #  P (base 0):  even rows, ky0+ky1 fused as K=128
                        #  Q (base 0):  odd rows, ky1+ky2 fused as K=128
                        #               plus ky0 singles (also base 0)
                        #  R (base 64): even rows, ky2 singles
                        pst = {
                            k: pspool.tile(
                                [OC, RG, W], F32, name=f"ps{k}", tag=f"ps{k}", bufs=2
                            )
                            for k in "PQR"
                        }
                        rlo = y0 // 2
                        rhi = y0 // 2 + 1
                        wts = wt_t[bi]
                        for kx in range(KW):
                            # P: K=128 pair (ky0 odd-half + ky1 even-half)
                            nc.tensor.matmul(
                                pst["P"][:, :, :],
                                wts[:, kx, :],
                                img[:, rlo : rlo + RG, kx : kx + W],
                                start=(kx == 0),
                                stop=(kx == KW - 1),
                            )
                            # Q: K=128 pair (ky1 odd-half + ky2 even-half)
                            nc.tensor.matmul(
                                pst["Q"][:, :, :],
                                wts[:, KW + kx, :],
                                img[:, rhi : rhi + RG, kx : kx + W],
                                start=(kx == 0),
                                stop=False,
                            )
                        for kx in range(KW):
                            # Q: ky0 singles (base 0, even half)
                            nc.tensor.matmul(
                                pst["Q"][:, :, :],
                                wts[0:64, 2 * KW + kx, :],
                                img[0:64, rlo : rlo + RG, kx : kx + W],
                                start=False,
                                stop=(kx == KW - 1),
                            )
                            # R: ky2 singles (base 64, odd half)
                            nc.tensor.matmul(
                                pst["R"][:, :, :],
                                wts[64:128, 2 * KW + kx, :],
                                img[64:128, rhi : rhi + RG, kx : kx + W],
                                start=(kx == 0),
                                stop=(kx == KW - 1),
                            )

                        osb = opool.tile(
                            [OC, RG, 2, W], F32, name="osb", tag="osb", bufs=3
                        )
                        nc.vector.tensor_scalar_add(
                            osb[:, :, 0, :], pst["P"][:, :, :], bias_t[:, bi : bi + 1]
                        )
                        nc.vector.tensor_add(
                            osb[:, :, 0, :], osb[:, :, 0, :], pst["R"][:, :, :]
                        )
                        nc.vector.tensor_scalar_add(
                            osb[:, :, 1, :], pst["Q"][:, :, :], bias_t[:, bi : bi + 1]
                        )
                        nc.scalar.dma_start(
                            o_d[bi, bj, :, y0 : y0 + H // GROUPS, :], osb[:, :, :, :]
                        )
    nc.compile()
    return nc


def _round_fp32r(a):
    """RNE-round fp32 to fp32r (11 stored mantissa bits; low 12 bits zero)."""
    u = a.view(np.uint32)
    r = (u + np.uint32(0x7FF) + ((u >> np.uint32(12)) & np.uint32(1))) & np.uint32(
        0xFFFFF000
    )
    return r.view(np.float32)


def _pack(x, weight, bias):
    """Host-side repack into the kernel's DMA-friendly layouts."""
    x = _round_fp32r(np.ascontiguousarray(x, dtype=np.float32))
    weight = _round_fp32r(np.ascontiguousarray(weight, dtype=np.float32))
    bias = np.ascontiguousarray(bias, dtype=np.float32)

    xp = np.zeros((B_I, B_J, 2, C, RH, WP), np.float32)
    xp[:, :, 0, :, 0:32, 1 : W + 1] = x[:, :, :, 0::2, :]   # even rows 0..62
    xp[:, :, 1, :, 1:33, 1 : W + 1] = x[:, :, :, 1::2, :]   # odd rows 1..63
    xp = xp.reshape(B_I, B_J, 2 * C, RH, WP)

    # wt0[bi, c, ky, kx, oc]
    wt0 = np.ascontiguousarray(np.transpose(weight, (0, 2, 3, 4, 1)))
    lo = np.concatenate([wt0[:, :, 1], wt0[:, :, 2], wt0[:, :, 0]], axis=2)
    hi = np.concatenate([wt0[:, :, 0], wt0[:, :, 1], wt0[:, :, 2]], axis=2)
    wt = np.concatenate([lo, hi], axis=1)  # [B_I, 2C, 9, OC]

    bp = np.ascontiguousarray(np.transpose(bias, (1, 0)))  # [OC, B_I]
    return xp, wt, bp


def make_in_maps(xp, wt, bp):
    in_maps = []
    for i in range(N_CORES):
        sl = slice(i * BPC, (i + 1) * BPC)
        in_maps.append(
            {
                "x": np.ascontiguousarray(xp[sl].reshape(NIMG, 2 * C, RH, WP)),
                "wt": np.ascontiguousarray(wt[sl]),
                "bias": np.ascontiguousarray(bp[:, sl]),
            }
        )
    return in_maps


def kernel(x, weight, bias):
    from concourse.bass_utils import run_bass_kernel_spmd

    xp, wt, bp = _pack(x, weight, bias)

    if "nc" not in _CACHE:
        _CACHE["nc"] = _build_nc()
    nc = _CACHE["nc"]

    in_maps = make_in_maps(xp, wt, bp)

    res = run_bass_kernel_spmd(nc, in_maps, list(range(N_CORES)))
    out = np.concatenate([res.results[i]["out"] for i in range(N_CORES)], axis=0)
    return out
